# revision 31
# baseline (speedup 1.0000x reference)
"""Trainium2 Bass kernel for nn_DiscreteDiffusionActionHead (v2).

Strategy: pure data-parallel over batch (B=8 -> 1 element per NeuronCore,
no collectives). Activations in [dim(partitions), token(free)] layout.
fp16 matmul inputs with fp32 PSUM accumulation.

v2 structural changes vs v1:
  - q/ks/ka projections head-packed along the psum free dim -> ONE rope
    op-set per projection instead of per head (8x fewer DVE ops).
  - psum->sbuf descale copies on the Scalar engine (ACT), freeing DVE.
  - softmax denominator via ones-matmul + reciprocal_approx_fast + one
    broadcast matmul (no slow 1-lane iterative divide).
  - LN rsqrt via fast-inverse-sqrt (DVE int ops + 2 Newton steps):
    no ACT table switches in the steady-state loop (exp set only).
  - Wo/Wf in fp16 (x-scaled), biases folded into matmuls (or skipped
    entirely when the staged inputs have all-zero biases, which they do).
  - ht+ha packed into one per-layer DMA; small per-layer consts packed.
"""
import numpy as np
import ml_dtypes

BF16 = ml_dtypes.bfloat16
F16 = np.float16
F8 = ml_dtypes.float8_e4m3
F32 = np.float32
WSCALE = 256.0

L_FULL = 24
D = 896
NH = 8
HD = 112
HP = 128
MQ = NH * HP            # 1024
KT = D // 128           # 7
T = 56
NVIS = 512
NADP = 64
NA = NADP + 1           # 65
NSA = T + NA            # 121 (self + adapter keys)
NQ = NH * T             # 448
NKA = NH * NA           # 520
VOCAB = 256
PD = 8
EPS = 1e-5
NCORES = 8
MAGIC = 0x5f3759df

# trig pack offsets (free-dim columns): cq|sq repeated per head, ca|sa, ct|st
O_CQ, O_SQ = 0, NQ
O_CA, O_SA = 2 * NQ, 2 * NQ + NKA
O_CT, O_ST = 2 * NQ + 2 * NKA, 2 * NQ + 2 * NKA + NVIS
TRIG_W = 2 * NQ + 2 * NKA + 2 * NVIS   # 2960

# bias16 pack rows: 0-7 bqk (q 0:112 | ks 112:224 | ka 224:336), 8 bkt,
# 9-11 bv (vs/va/vt), 12 bo
B16_W = 1024
# bpk (f32) columns: ln_g (7), ln_b (7), bf (7)
B_G, B_B, B_F = 0, 7, 14
NBP = 21


# ----------------------------------------------------------------------------
# host-side layout helpers
# ----------------------------------------------------------------------------

def _rope_tables(n):
    inv = 1.0 / (10000.0 ** (np.arange(0, HD, 2, dtype=F32) / HD))
    f = np.arange(n, dtype=F32)[:, None] * inv[None, :]
    emb = np.concatenate([f, f], axis=-1)               # (n, 112)
    return np.cos(emb), np.sin(emb)


def _trig_pad(n, rep):
    c, s = _rope_tables(n)
    cp = np.zeros((HP, n * rep), F32)
    sp = np.zeros((HP, n * rep), F32)
    cp[:HD] = np.tile(c.T, (1, rep))
    sp[:HD] = np.tile(s.T, (1, rep))
    return cp, sp


def _pad_cols(W):
    Wp = np.zeros((W.shape[0], MQ), F32)
    for h in range(NH):
        Wp[:, HP * h:HP * h + HD] = W[:, HD * h:HD * h + HD]
    return Wp


def _pad_rows(W):
    Wp = np.zeros((MQ, W.shape[1]), F32)
    for h in range(NH):
        Wp[HP * h:HP * h + HD, :] = W[HD * h:HD * h + HD, :]
    return Wp


def _lhsT(W, dtype=F16):
    """[Din, M] -> [128, Din//128, M] sbuf layout."""
    Din, M = W.shape
    return np.ascontiguousarray(
        W.reshape(Din // 128, 128, M).transpose(1, 0, 2)).astype(dtype)


def _pk(b):
    """per-partition bias pack: [nm*128] -> [128, nm]"""
    nm = b.shape[0] // 128
    return np.ascontiguousarray(b.reshape(nm, 128).T).astype(F32)


def _shift_T():
    S = np.zeros((HP, HP), F32)
    for i in range(HD // 2):
        S[2 * i, 2 * i + 1] = -1.0
        S[2 * i + 1, 2 * i] = 1.0
    return np.ascontiguousarray(S.T).astype(F16)


def prep_shared(inp, L):
    """Layout transforms shared by all cores (weights etc)."""
    g = {}
    for k, v in inp.items():
        a = np.asarray(v)
        g[k] = a if np.issubdtype(a.dtype, np.integer) else a.astype(F32)
    scale = F32(1.0 / np.sqrt(HD))
    rg = np.tanh(g['gate'])                      # [L]

    zb = all(float(np.abs(g[k]).max()) == 0.0 for k in
             ('bq', 'bks', 'bka', 'bkt', 'bvs', 'bva', 'bvt', 'bo'))

    wq = np.empty((L, 128, KT, MQ), F16)
    wks = np.empty((L, 128, KT, MQ), F16)
    wka = np.empty((L, 128, KT, MQ), F16)
    wkt = np.empty((L, 128, KT, MQ), F16)
    wvs = np.empty((L, 128, KT, D), F16)
    wva = np.empty((L, 128, KT, D), F16)
    wvt = np.empty((L, 128, KT, D), F16)
    wo = np.empty((L, 128, NH, D), F16)
    wf = np.empty((L, 128, KT, D), F32)
    b16 = np.zeros((L, 13, B16_W), F16)
    bpk = np.empty((L, 128, NBP), F32)

    for l in range(L):
        wq[l] = _lhsT(_pad_cols(g['Wq'][l] * (scale * WSCALE)))
        wks[l] = _lhsT(_pad_cols(g['Wks'][l] * WSCALE))
        wka[l] = _lhsT(_pad_cols(g['Wka'][l] * WSCALE))
        wkt[l] = _lhsT(_pad_cols(g['Wkt'][l] * (rg[l] * WSCALE)))
        wvs[l] = _lhsT(g['Wvs'][l] * WSCALE)
        wva[l] = _lhsT(g['Wva'][l] * WSCALE)
        wvt[l] = _lhsT(g['Wvt'][l] * WSCALE)
        wo[l] = _lhsT(_pad_rows(g['Wo'][l] * WSCALE))
        wf[l] = _lhsT(g['Wf'][l], F32)
        b16[l, 0:8, 0:HD] = (g['bq'][l] * (scale * WSCALE)).reshape(NH, HD)
        b16[l, 0:8, HD:2 * HD] = (g['bks'][l] * WSCALE).reshape(NH, HD)
        b16[l, 0:8, 2 * HD:3 * HD] = (g['bka'][l] * WSCALE).reshape(NH, HD)
        b16[l, 8, 0:D] = g['bkt'][l] * (rg[l] * WSCALE)
        b16[l, 9, 0:D] = g['bvs'][l] * WSCALE
        b16[l, 10, 0:D] = g['bva'][l] * WSCALE
        b16[l, 11, 0:D] = g['bvt'][l] * WSCALE
        b16[l, 12, 0:D] = g['bo'][l] * WSCALE
        bpk[l, :, B_G:B_G + 7] = _pk(g['ln_g'][l])
        bpk[l, :, B_B:B_B + 7] = _pk(g['ln_b'][l])
        bpk[l, :, B_F:B_F + 7] = _pk(g['bf'][l])

    trig = np.empty((128, TRIG_W), F32)
    trig[:, O_CQ:O_CQ + NQ], trig[:, O_SQ:O_SQ + NQ] = _trig_pad(T, NH)
    trig[:, O_CA:O_CA + NKA], trig[:, O_SA:O_SA + NKA] = _trig_pad(NA, NH)
    trig[:, O_CT:O_CT + NVIS], trig[:, O_ST:O_ST + NVIS] = _trig_pad(NVIS, 1)

    hsel = np.zeros((8, NQ + NKA), F16)
    for h in range(NH):
        hsel[h, T * h:T * h + T] = 1.0
        hsel[h, NQ + NA * h:NQ + NA * h + NA] = 1.0

    fin = np.zeros((128, 16), F32)
    fin[:, 0:7] = _pk(g['og'])
    fin[:, 7:14] = _pk(g['ob'])
    fin[:, 14:16] = _pk(g['bout'])

    shared = {
        'wq': wq, 'wks': wks, 'wka': wka, 'wkt': wkt,
        'wvs': wvs, 'wva': wva, 'wvt': wvt, 'wo': wo, 'wf': wf,
        'wout': _lhsT(g['Wout'], F32),
        'b16': b16, 'bpk': bpk,
        'trig': trig.astype(F16), 'hsel': hsel, 'fin': fin,
        'shift_t': _shift_T(),
    }
    return shared, g, zb


def prep_core(g, b, L):
    """Per-core (= per batch element) activations in sbuf layout."""
    mhs = g['multi_layer_hidden_states']
    p = g['proprio'][b] @ g['Wp'] + g['bp']                    # [D]
    hta = np.empty((L, 128, KT, NVIS + NA), F16)
    for l in range(L):
        ht = mhs[b, l, :NVIS, :].T.reshape(KT, 128, NVIS)
        ha = np.concatenate([mhs[b, l, NVIS:, :], p[None]], 0).T \
            .reshape(KT, 128, NA)
        hta[l] = np.concatenate([ht, ha], 2).transpose(1, 0, 2)
    x0 = g['tok_emb'][np.asarray(g['input_tokens'][b], np.int64)].T   # [D, T]
    x0 = np.ascontiguousarray(x0.reshape(KT, 128, T).transpose(1, 0, 2)).astype(F32)
    return {'hta': hta, 'x0': x0}


# ----------------------------------------------------------------------------
# bass program
# ----------------------------------------------------------------------------

def build_program(L, xdbg=False, zb=True, variant=None):
    import itertools
    _ctr = itertools.count()
    import concourse.tile as tile
    import concourse.mybir as mybir
    from concourse import bacc

    dt = mybir.dt
    AF = mybir.ActivationFunctionType
    OP = mybir.AluOpType
    DS = 1.0 / WSCALE

    nc = bacc.Bacc("TRN2", target_bir_lowering=False, debug=False,
                   num_devices=NCORES, name="ddah2")

    def din(name, shape, dtype=dt.float16):
        return nc.dram_tensor(name, shape, dtype, kind="ExternalInput")

    d_wq = din("wq", [L, 128, KT, MQ])
    d_wks = din("wks", [L, 128, KT, MQ])
    d_wka = din("wka", [L, 128, KT, MQ])
    d_wkt = din("wkt", [L, 128, KT, MQ])
    d_wvs = din("wvs", [L, 128, KT, D])
    d_wva = din("wva", [L, 128, KT, D])
    d_wvt = din("wvt", [L, 128, KT, D])
    d_wo = din("wo", [L, 128, NH, D])
    d_wf = din("wf", [L, 128, KT, D], dt.float32)
    d_wout = din("wout", [128, KT, VOCAB], dt.float32)
    d_b16 = None if zb else din("b16", [L, 13, B16_W])
    d_bpk = din("bpk", [L, 128, NBP], dt.float32)
    d_trig = din("trig", [128, TRIG_W])
    d_hsel = None if zb else din("hsel", [8, NQ + NKA])
    d_fin = din("fin", [128, 16], dt.float32)
    d_shift = din("shift_t", [128, 128])
    d_hta = din("hta", [L, 128, KT, NVIS + NA])
    d_x0 = din("x0", [128, KT, T], dt.float32)
    d_out = nc.dram_tensor("out", [128, 2, T], dt.float32, kind="ExternalOutput")
    taps = {}

    def tap(name, tile_, l=0):
        if not xdbg or l != 0 or name in taps:
            return
        dtt = nc.dram_tensor(f"tap_{name}", list(tile_.shape), tile_.dtype,
                             kind="ExternalOutput")
        taps[name] = dtt
        nc.sync.dma_start(dtt[:], tile_)
    d_xdbg = None
    if xdbg:
        d_xdbg = nc.dram_tensor("xdbg", [L, 128, KT, T], dt.float32,
                                kind="ExternalOutput")

    with tile.TileContext(nc) as tc, \
         tc.tile_pool(name="singles", bufs=1) as singles, \
         tc.tile_pool(name="wp", bufs=6) as wp, \
         tc.tile_pool(name="wp32", bufs=2) as wp32, \
         tc.tile_pool(name="wpo", bufs=2) as wpo, \
         tc.tile_pool(name="iop", bufs=2) as iop, \
         tc.tile_pool(name="kvp", bufs=2) as kvp, \
         tc.tile_pool(name="tmp", bufs=2) as tmp, \
         tc.tile_pool(name="att", bufs=2) as att, \
         tc.tile_pool(name="xp", bufs=2) as xp, \
         tc.tile_pool(name="yp", bufs=2) as yp, \
         tc.tile_pool(name="st", bufs=2) as st, \
         tc.tile_pool(name="psA", bufs=4, space="PSUM") as psA, \
         tc.tile_pool(name="psB", bufs=2, space="PSUM") as psB, \
         tc.tile_pool(name="psV", bufs=1, space="PSUM") as psV:

        # ---- constants loaded once ----
        trig = singles.tile([128, TRIG_W], dt.float16, name="trig")
        nc.sync.dma_start(trig, d_trig[:])
        shift = singles.tile([128, 128], dt.float16, name="shift")
        nc.sync.dma_start(shift, d_shift[:])
        hsel = ones16 = None
        if not zb:
            hsel = singles.tile([8, NQ + NKA], dt.float16, name="hsel")
            nc.sync.dma_start(hsel, d_hsel[:])
            ones16 = singles.tile([1, NVIS], dt.float16, name="ones16")
            nc.vector.memset(ones16, 1.0)
        fin = singles.tile([128, 16], dt.float32, name="fin")
        nc.sync.dma_start(fin, d_fin[:])
        ones_h = singles.tile([128, 1], dt.float16, name="ones_h")
        nc.vector.memset(ones_h, 1.0)
        ones_f = singles.tile([128, 1], dt.float32, name="ones_f")
        nc.vector.memset(ones_f, 1.0)
        ones_r32 = singles.tile([1, 128], dt.float32, name="ones_r32")
        nc.vector.memset(ones_r32, 1.0)
        magic = singles.tile([1, T], dt.int32, name="magic")
        nc.vector.memset(magic, MAGIC)

        cq, sq = trig[:, O_CQ:O_CQ + NQ], trig[:, O_SQ:O_SQ + NQ]
        ca, sa = trig[:, O_CA:O_CA + NKA], trig[:, O_SA:O_SA + NKA]
        ct, stg = trig[:, O_CT:O_CT + NVIS], trig[:, O_ST:O_ST + NVIS]

        x_sb = xp.tile([128, KT, T], dt.float32, tag="x", name="x_init")
        nc.sync.dma_start(x_sb, d_x0[:])

        def load_wh(dram, l, k2, m):
            """Load a [128, k2, m] fp16 weight in two column halves."""
            halves = []
            for cols in (slice(0, 512), slice(512, m)):
                n = cols.stop - cols.start
                w = wp.tile([128, k2, 512], dt.float16, tag="w",
                            name=f"t{next(_ctr)}")[:, :, :n]
                nc.sync.dma_start(w, dram[l][:, :, cols])
                halves.append(w)
            return halves

        def rsqrt56(out, vin, pf):
            """out[1,T] = 1/sqrt(vin[1,T]) via fast-inverse-sqrt + 2 NR."""
            ti = st.tile([1, T], dt.int32, tag="rs_i", name=f"t{next(_ctr)}")
            nc.vector.tensor_scalar(ti, vin.bitcast(dt.int32), 1, None,
                                    OP.logical_shift_right)
            yi = st.tile([1, T], dt.int32, tag="rs_y", name=f"t{next(_ctr)}")
            nc.vector.tensor_tensor(yi, magic, ti, OP.subtract)
            y = yi.bitcast(dt.float32)
            vh = st.tile([1, T], dt.float32, tag="rs_vh", name=f"t{next(_ctr)}")
            nc.vector.tensor_scalar_mul(vh, vin, -0.5)
            for it in range(2):
                z = st.tile([1, T], dt.float32, tag=f"rs_z{it}",
                            name=f"t{next(_ctr)}")
                nc.vector.tensor_tensor(z, y, y, OP.mult)
                nc.vector.tensor_tensor(z, z, vh, OP.mult)
                nc.vector.tensor_scalar(z, z, 1.5, None, OP.add)
                yn = st.tile([1, T], dt.float32, tag=f"rs_y{it}",
                             name=f"t{next(_ctr)}") if it == 0 else out
                nc.vector.tensor_tensor(yn, y, z, OP.mult)
                y = yn

        def qk_pack(w_sb, rhs_sb, rhs_cols, nh2, ntok, h0, bias_lhs, bias_rhs,
                    cosr, sinr, out_view):
            """Head-packed projection + rope.
            psum [128, nh2*ntok]; heads h0..h0+nh2; writes rope'd fp16 to
            out_view (any AP with matching free size)."""
            width = nh2 * ntok
            ps = psA.tile([128, 512], dt.float32, tag="psA",
                          name=f"t{next(_ctr)}")[:, :width]
            if not zb:
                nc.tensor.matmul(ps, bias_lhs, bias_rhs, start=True, stop=False)
            for hh in range(nh2):
                h = h0 + hh
                wh = w_sb[h // 4]
                hc = (h % 4) * HP
                cs = slice(hh * ntok, hh * ntok + ntok)
                for k in range(KT):
                    nc.tensor.matmul(ps[:, cs], wh[:, k, hc:hc + HP],
                                     rhs_sb[:, k, rhs_cols],
                                     start=(zb and k == 0), stop=(k == KT - 1))
            sb = tmp.tile([128, 512], dt.float16, tag="qksb",
                          name=f"t{next(_ctr)}")[:, :width]
            nc.scalar.mul(sb, ps, DS)
            rps = psB.tile([128, 512], dt.float32, tag="psB",
                           name=f"t{next(_ctr)}")[:, :width]
            nc.tensor.matmul(rps, shift, sb, start=True, stop=True)
            t1 = tmp.tile([128, 512], dt.float16, tag="qkt1",
                          name=f"t{next(_ctr)}")[:, :width]
            nc.vector.tensor_tensor(t1, sb, cosr, OP.mult)
            t2 = tmp.tile([128, 512], dt.float16, tag="qkt2",
                          name=f"t{next(_ctr)}")[:, :width]
            nc.vector.tensor_tensor(t2, rps, sinr, OP.mult)
            nc.vector.tensor_tensor(out_view, t1, t2, OP.add)

        def proj_v(w_sb, act_sb, act_cols, mtok, bias_row, b16t, out_view):
            """[tokens, D] projection: out_view [mtok, D] fp16."""
            ps = psV.tile([128, D], dt.float32, tag="psV",
                          name=f"t{next(_ctr)}")[:mtok]
            for hi, sl in enumerate((slice(0, 512), slice(512, D))):
                wh = w_sb[hi]
                if not zb:
                    nc.tensor.matmul(ps[:, sl], ones16[0:1, :mtok],
                                     b16t[bias_row:bias_row + 1, sl],
                                     start=True, stop=False)
                for k in range(KT):
                    nc.tensor.matmul(ps[:, sl], act_sb[:, k, act_cols],
                                     wh[:, k, :],
                                     start=(zb and k == 0), stop=(k == KT - 1))
            nc.scalar.mul(out_view, ps, DS)

        nrep = 1
        if isinstance(variant, tuple) and variant[0] == "rep":
            nrep = variant[1]
        for li in range(L * nrep):
            l = li % L
            bsb = st.tile([128, NBP], dt.float32, tag="bias",
                          name=f"t{next(_ctr)}")
            nc.gpsimd.dma_start(bsb, d_bpk[l])
            b16t = None
            if not zb:
                b16t = st.tile([13, B16_W], dt.float16, tag="b16",
                               name=f"t{next(_ctr)}")
                nc.sync.dma_start(b16t, d_b16[l])
            hta = iop.tile([128, KT, NVIS + NA], dt.float16, tag="hta",
                           name=f"t{next(_ctr)}")
            nc.gpsimd.dma_start(hta, d_hta[l])
            sl_ht = slice(0, NVIS)
            sl_ha = slice(NVIS, NVIS + NA)

            # fp16 shadow of the fp32 residual stream
            x16 = xp.tile([128, KT, T], dt.float16, tag="x16",
                          name=f"t{next(_ctr)}")
            nc.vector.tensor_copy(out=x16, in_=x_sb)

            # ---- kt projection + rope (per head; full psum banks) ----
            w = load_wh(d_wkt, l, KT, MQ)
            ktr = kvp.tile([128, NH, NVIS], dt.float16, tag="ktr",
                           name=f"t{next(_ctr)}")
            for h in range(NH):
                wh = w[h // 4]
                hc = (h % 4) * HP
                ps = psA.tile([128, 512], dt.float32, tag="psA",
                              name=f"t{next(_ctr)}")
                if not zb:
                    nc.tensor.matmul(ps, b16t[8:9, HD * h:HD * h + HD],
                                     ones16[0:1, :NVIS], start=True, stop=False)
                for k in range(KT):
                    nc.tensor.matmul(ps, wh[:, k, hc:hc + HP],
                                     hta[:, k, sl_ht],
                                     start=(zb and k == 0), stop=(k == KT - 1))
                ksb = tmp.tile([128, NVIS], dt.float16, tag="ktsb",
                               name=f"t{next(_ctr)}")
                nc.scalar.mul(ksb, ps, DS)
                rps = psB.tile([128, 512], dt.float32, tag="psB",
                               name=f"t{next(_ctr)}")
                nc.tensor.matmul(rps, shift, ksb, start=True, stop=True)
                t1 = tmp.tile([128, NVIS], dt.float16, tag="ktt1",
                              name=f"t{next(_ctr)}")
                nc.vector.tensor_tensor(t1, ksb, ct, OP.mult)
                t2 = tmp.tile([128, NVIS], dt.float16, tag="ktt2",
                              name=f"t{next(_ctr)}")
                nc.vector.tensor_tensor(t2, rps, stg, OP.mult)
                nc.vector.tensor_tensor(ktr[:, h, :], t1, t2, OP.add)
            tap("ktr", ktr, l)

            # ---- vt ----
            w = load_wh(d_wvt, l, KT, D)
            vt_sb = kvp.tile([128, 4, D], dt.float16, tag="vt",
                             name=f"t{next(_ctr)}")
            for m in range(4):
                proj_v(w, hta, slice(128 * m, 128 * m + 128), 128, 11, b16t,
                       vt_sb[:, m, :])
            tap("vt", vt_sb, l)

            # ---- vs / va ----
            vs_t = kvp.tile([T, D], dt.float16, tag="vs",
                            name=f"t{next(_ctr)}")
            w = load_wh(d_wvs, l, KT, D)
            proj_v(w, x16, slice(0, T), T, 9, b16t, vs_t)
            va_t = kvp.tile([NA, D], dt.float16, tag="va",
                            name=f"t{next(_ctr)}")
            w = load_wh(d_wva, l, KT, D)
            proj_v(w, hta, sl_ha, NA, 10, b16t, va_t)

            # ---- q / ks / ka (head-packed) + rope ----
            ksa = kvp.tile([128, NH, NSA], dt.float16, tag="ksa",
                           name=f"t{next(_ctr)}")
            w = load_wh(d_wq, l, KT, MQ)
            qro = kvp.tile([128, NQ], dt.float16, tag="qro",
                           name=f"t{next(_ctr)}")
            qk_pack(w, x16, slice(0, T), NH, T, 0,
                    None if zb else b16t[0:8, 0:HD],
                    None if zb else hsel[:, :NQ], cq, sq, qro)
            tap("qro", qro, l)
            w = load_wh(d_wks, l, KT, MQ)
            qk_pack(w, x16, slice(0, T), NH, T, 0,
                    None if zb else b16t[0:8, HD:2 * HD],
                    None if zb else hsel[:, :NQ], cq, sq,
                    ksa[:, :, :T])
            w = load_wh(d_wka, l, KT, MQ)
            for half in range(2):
                h0 = 4 * half
                cslice = slice(NA * h0, NA * h0 + 4 * NA)
                qk_pack(w, hta, sl_ha, 4, NA, h0,
                        None if zb else b16t[h0:h0 + 4, 2 * HD:3 * HD],
                        None if zb else
                        hsel[h0:h0 + 4, NQ + NA * h0:NQ + NA * h0 + 4 * NA],
                        ca[:, cslice], sa[:, cslice], ksa[:, h0:h0 + 4, T:NSA])

            # ---- scores (transposed: [keys, h*q]) + exp ----
            ex_s = att.tile([T, NQ], dt.float16, tag="exs",
                            name=f"t{next(_ctr)}")
            ps = psA.tile([128, 512], dt.float32, tag="psA",
                          name=f"t{next(_ctr)}")[:T, :NQ]
            for h in range(NH):
                nc.tensor.matmul(ps[:, T * h:T * h + T], ksa[:, h, :T],
                                 qro[:, T * h:T * h + T], start=True, stop=True)
            nc.scalar.activation(ex_s, ps, AF.Exp)
            tap("ex_s", ex_s, l)
            ex_a = att.tile([NA, NQ], dt.float16, tag="exa",
                            name=f"t{next(_ctr)}")
            ps = psA.tile([128, 512], dt.float32, tag="psA",
                          name=f"t{next(_ctr)}")[:NA, :NQ]
            for h in range(NH):
                nc.tensor.matmul(ps[:, T * h:T * h + T], ksa[:, h, T:NSA],
                                 qro[:, T * h:T * h + T], start=True, stop=True)
            nc.scalar.activation(ex_a, ps, AF.Exp)
            ex_t = att.tile([128, 4, NQ], dt.float16, tag="ext",
                            name=f"t{next(_ctr)}")
            for m in range(4):
                ps = psA.tile([128, 512], dt.float32, tag="psA",
                              name=f"t{next(_ctr)}")[:, :NQ]
                for h in range(NH):
                    nc.tensor.matmul(ps[:, T * h:T * h + T],
                                     ktr[:, h, 128 * m:128 * m + 128],
                                     qro[:, T * h:T * h + T],
                                     start=True, stop=True)
                nc.scalar.activation(ex_t[:, m, :], ps, AF.Exp)

            # ---- softmax denom ----
            lps = psB.tile([1, 512], dt.float32, tag="psB",
                           name=f"t{next(_ctr)}")[:, :NQ]
            nc.tensor.matmul(lps, ones_h[:T], ex_s, start=True, stop=False)
            nc.tensor.matmul(lps, ones_h[:NA], ex_a, start=False, stop=False)
            for m in range(4):
                nc.tensor.matmul(lps, ones_h, ex_t[:, m, :],
                                 start=False, stop=(m == 3))
            linv = st.tile([1, NQ], dt.float32, tag="linv",
                           name=f"t{next(_ctr)}")
            nc.vector.reciprocal_approx_fast(out=linv, in_=lps)
            tap("linv", linv, l)
            lrep_ps = psB.tile([128, 512], dt.float32, tag="psB",
                               name=f"t{next(_ctr)}")[:, :NQ]
            nc.tensor.matmul(lrep_ps, ones_r32, linv, start=True, stop=True)
            lrep = att.tile([128, NQ], dt.float32, tag="lrep",
                            name=f"t{next(_ctr)}")
            nc.scalar.mul(lrep, lrep_ps, 1.0)

            # ---- attn @ V (o normalized) ----
            o16 = att.tile([128, NH, T], dt.float16, tag="o16",
                           name=f"t{next(_ctr)}")
            nc.vector.memset(o16[96:128], 0.0)
            for h in range(NH):
                hs = slice(HD * h, HD * h + HD)
                ops = psA.tile([128, 512], dt.float32, tag="psA",
                               name=f"t{next(_ctr)}")[:HD, :T]
                nc.tensor.matmul(ops, vs_t[:, hs], ex_s[:, T * h:T * h + T],
                                 start=True, stop=False)
                nc.tensor.matmul(ops, va_t[:, hs], ex_a[:, T * h:T * h + T],
                                 start=False, stop=False)
                for m in range(4):
                    nc.tensor.matmul(ops, vt_sb[:, m, hs],
                                     ex_t[:, m, T * h:T * h + T],
                                     start=False, stop=(m == 3))
                nc.vector.tensor_tensor(o16[:HD, h, :], ops,
                                        lrep[:HD, T * h:T * h + T], OP.mult)
            tap("o16", o16, l)

            # ---- Wo + residual ----
            y_sb = yp.tile([128, KT, T], dt.float32, tag="y",
                           name=f"t{next(_ctr)}")
            for half, mos in ((0, range(4)), (1, range(4, KT))):
                cols = slice(0, 512) if half == 0 else slice(512, D)
                ncols = 512 if half == 0 else D - 512
                w = wpo.tile([128, NH, 512], dt.float16, tag="wo",
                             name=f"t{next(_ctr)}")[:, :, :ncols]
                nc.scalar.dma_start(w, d_wo[l][:, :, cols])
                for mo in mos:
                    mc = mo * 128 - half * 512
                    ps = psA.tile([128, 512], dt.float32, tag="psA",
                                  name=f"t{next(_ctr)}")[:, :T]
                    if not zb:
                        nc.tensor.matmul(ps, b16t[12:13, 128 * mo:128 * mo + 128],
                                         ones16[0:1, :T], start=True, stop=False)
                    for k in range(NH):
                        nc.tensor.matmul(ps, w[:, k, mc:mc + 128],
                                         o16[:, k, :],
                                         start=(zb and k == 0), stop=(k == NH - 1))
                    nc.vector.scalar_tensor_tensor(
                        y_sb[:, mo, :], ps, DS, x_sb[:, mo, :], OP.mult, OP.add)
            tap("y", y_sb, l)

            # ---- layernorm ----
            mps = psB.tile([1, 512], dt.float32, tag="psB",
                           name=f"t{next(_ctr)}")[:, :2 * T]
            ysq = yp.tile([128, KT, T], dt.float32, tag="ysq",
                          name=f"t{next(_ctr)}")
            nc.scalar.activation(ysq, y_sb, AF.Square)
            for k in range(KT):
                nc.tensor.matmul(mps[:, :T], ones_f, y_sb[:, k, :],
                                 start=(k == 0), stop=(k == KT - 1))
            for k in range(KT):
                nc.tensor.matmul(mps[:, T:], ones_f, ysq[:, k, :],
                                 start=(k == 0), stop=(k == KT - 1))
            rcp = st.tile([1, 2 * T], dt.float32, tag="rcp",
                          name=f"t{next(_ctr)}")
            mean, var = rcp[:, :T], rcp[:, T:]
            nc.vector.tensor_scalar_mul(mean, mps[:, :T], 1.0 / D)
            msq = st.tile([1, T], dt.float32, tag="msq", name=f"t{next(_ctr)}")
            nc.vector.tensor_tensor(msq, mean, mean, OP.mult)
            nc.vector.tensor_scalar(msq, msq, EPS, None, OP.subtract)
            nc.vector.scalar_tensor_tensor(var, mps[:, T:], 1.0 / D, msq,
                                           OP.mult, OP.subtract)
            rc2 = st.tile([1, 2 * T], dt.float32, tag="rc2",
                          name=f"t{next(_ctr)}")
            rsqrt56(rc2[:, :T], var, l)
            nc.vector.tensor_tensor(rc2[:, T:], mean, rc2[:, :T], OP.mult)
            rrep = psB.tile([128, 512], dt.float32, tag="psB",
                            name=f"t{next(_ctr)}")[:, :2 * T]
            nc.tensor.matmul(rrep, ones_r32, rc2, start=True, stop=True)
            lnf = yp.tile([128, KT, T], dt.float32, tag="lnf",
                          name=f"t{next(_ctr)}")
            for k in range(KT):
                t1 = tmp.tile([128, T], dt.float32, tag="lnt1",
                              name=f"t{next(_ctr)}")
                nc.vector.tensor_tensor(t1, y_sb[:, k, :], rrep[:, :T], OP.mult)
                nc.vector.tensor_tensor(t1, t1, rrep[:, T:], OP.subtract)
                nc.vector.tensor_scalar(lnf[:, k, :], t1,
                                        bsb[:, B_G + k:B_G + k + 1],
                                        bsb[:, B_B + k:B_B + k + 1],
                                        OP.mult, OP.add)
            tap("ln", lnf, l)

            # ---- Wf (fp32) + relu -> new x ----
            x_new = xp.tile([128, KT, T], dt.float32, tag="x",
                            name=f"t{next(_ctr)}")
            for half, mos in ((0, range(4)), (1, range(4, KT))):
                cols = slice(0, 512) if half == 0 else slice(512, D)
                ncols = 512 if half == 0 else D - 512
                w32 = wp32.tile([128, KT, 512], dt.float32, tag="w32",
                                name=f"t{next(_ctr)}")[:, :, :ncols]
                nc.scalar.dma_start(w32, d_wf[l][:, :, cols])
                for mo in mos:
                    mc = mo * 128 - half * 512
                    ps = psA.tile([128, 512], dt.float32, tag="psA",
                                  name=f"t{next(_ctr)}")[:, :T]
                    for k in range(KT):
                        nc.tensor.matmul(ps, w32[:, k, mc:mc + 128],
                                         lnf[:, k, :],
                                         start=(k == 0), stop=(k == KT - 1))
                    nc.scalar.activation(x_new[:, mo, :], ps, AF.Relu,
                                         bias=bsb[:, B_F + mo:B_F + mo + 1])
            x_sb = x_new
            if d_xdbg is not None and li == l:
                nc.sync.dma_start(d_xdbg[l], x_sb)

        # ---- final layernorm + Wout (fp32) ----
        mps = psB.tile([1, 512], dt.float32, tag="psB",
                       name=f"t{next(_ctr)}")[:, :2 * T]
        ysq = yp.tile([128, KT, T], dt.float32, tag="ysq", name=f"t{next(_ctr)}")
        nc.scalar.activation(ysq, x_sb, AF.Square)
        for k in range(KT):
            nc.tensor.matmul(mps[:, :T], ones_f, x_sb[:, k, :],
                             start=(k == 0), stop=(k == KT - 1))
        for k in range(KT):
            nc.tensor.matmul(mps[:, T:], ones_f, ysq[:, k, :],
                             start=(k == 0), stop=(k == KT - 1))
        rcp = st.tile([1, 2 * T], dt.float32, tag="rcp", name=f"t{next(_ctr)}")
        mean, var = rcp[:, :T], rcp[:, T:]
        nc.vector.tensor_scalar_mul(mean, mps[:, :T], 1.0 / D)
        msq = st.tile([1, T], dt.float32, tag="msq", name=f"t{next(_ctr)}")
        nc.vector.tensor_tensor(msq, mean, mean, OP.mult)
        nc.vector.tensor_scalar(msq, msq, EPS, None, OP.subtract)
        nc.vector.scalar_tensor_tensor(var, mps[:, T:], 1.0 / D, msq,
                                       OP.mult, OP.subtract)
        rc2 = st.tile([1, 2 * T], dt.float32, tag="rc2", name=f"t{next(_ctr)}")
        rsqrt56(rc2[:, :T], var, -1)
        nc.vector.tensor_tensor(rc2[:, T:], mean, rc2[:, :T], OP.mult)
        rrep = psB.tile([128, 512], dt.float32, tag="psB",
                        name=f"t{next(_ctr)}")[:, :2 * T]
        nc.tensor.matmul(rrep, ones_r32, rc2, start=True, stop=True)
        lnf = yp.tile([128, KT, T], dt.float32, tag="lnf", name=f"t{next(_ctr)}")
        for k in range(KT):
            t1 = tmp.tile([128, T], dt.float32, tag="lnt1", name=f"t{next(_ctr)}")
            nc.vector.tensor_tensor(t1, x_sb[:, k, :], rrep[:, :T], OP.mult)
            nc.vector.tensor_tensor(t1, t1, rrep[:, T:], OP.subtract)
            nc.vector.tensor_scalar(lnf[:, k, :], t1,
                                    fin[:, k:k + 1], fin[:, 7 + k:8 + k],
                                    OP.mult, OP.add)
        woutsb = wp32.tile([128, KT, 512], dt.float32, tag="w32",
                           name="woutsb")[:, :, :VOCAB]
        nc.scalar.dma_start(woutsb, d_wout[:])
        out_sb = yp.tile([128, 2, T], dt.float32, tag="outsb",
                         name=f"t{next(_ctr)}")
        for mo in range(2):
            ps = psA.tile([128, 512], dt.float32, tag="psA",
                          name=f"t{next(_ctr)}")[:, :T]
            for k in range(KT):
                nc.tensor.matmul(ps, woutsb[:, k, 128 * mo:128 * mo + 128],
                                 lnf[:, k, :],
                                 start=(k == 0), stop=(k == KT - 1))
            nc.vector.tensor_scalar_add(out_sb[:, mo, :], ps,
                                        fin[:, 14 + mo:15 + mo])
        nc.sync.dma_start(d_out[:], out_sb)

    nc.compile()
    return nc


_PROG_CACHE = {}


def _get_program(L, xdbg=False, zb=True, variant=None):
    key = (L, xdbg, zb, variant)
    if key not in _PROG_CACHE:
        _PROG_CACHE[key] = build_program(L, xdbg, zb, variant)
    return _PROG_CACHE[key]


def run(inputs, L=L_FULL, xdbg=False):
    from concourse.bass_utils import run_bass_kernel_spmd
    shared, g, zb = prep_shared(inputs, L)
    nc = _get_program(L, xdbg, zb)
    in_maps = []
    for b in range(NCORES):
        m = dict(shared)
        m.update(prep_core(g, b, L))
        in_maps.append(m)
    res = run_bass_kernel_spmd(nc, in_maps, core_ids=list(range(NCORES)))
    outs = []
    for r in res.results:
        o = r["out"]                                    # [128, 2, T]
        outs.append(np.ascontiguousarray(o.transpose(2, 1, 0)).reshape(T, VOCAB))
    full = np.stack(outs).astype(F32)                   # [B, T, VOCAB]
    if xdbg:
        return full, res.results
    return full


def kernel(**inputs) -> np.ndarray:
    return run(inputs, L=L_FULL)
